# revision 1
# baseline (speedup 1.0000x reference)
"""Causal single-head attention (B=4, S=4096, D=1024, H=128) on 8 NeuronCores.

Sharding: core c = (batch b = c//2, half h = c%2). Each core:
  - computes K^T [h, 4096] and V [4096, 4] for its full batch row (replicated
    across the 2 cores of a batch),
  - handles 2048 query rows: 16 parity-interleaved 128-row subtiles
    (global subtile g = 8*r + 2*s + h for slot r in 0..3, s in 0..3),
  - slots have uniform causal k-tile limits [8, 16, 24, 32] so all 8 cores run
    the identical compiled program; causality is enforced with per-core mask
    DATA (qpos vs kiota is_ge compare) on the last 8 k-iters of each slot.

Pipeline per core (all matmuls bf16 with fp32 PSUM accumulate):
  x^T, xq^T arrive host-transposed in bf16; plain contiguous DMA loads issued
    column-range-major so projections start after the first 512 columns
  K^T/V^T/Q^T = W.T @ x^T, weight-stationary over stripe pairs
    (+bias via ACT Identity on the PSUM->SBUF copy)
  V natural via PE transpose of V^T into a shared-tag PSUM slot + DVE copy
  attention in two kt-outer passes over slot pairs (K/V bricks loaded once
  per kt, two same-weight matmuls), software-pipelined one kt deep:
    S^T[k, q] = K-brick.T @ Q^T  (contraction h=128, N<=512, h-safe
      column narrowing skips bricks masked for both core halves)
    P^T = exp(S^T / sqrt(H))  (ACT, bf16 out; no max subtraction - scores +-2.5)
    causal mask fused: P^T = (qpos >= kpos) * P^T  (DVE scalar_tensor_tensor,
      int16 positions) on the last-8 k-iters of each slot
    O^T[h, q] += V-brick.T @ P^T  (PSUM accumulate)
    d partials accumulated on DVE in bf16; one ones-matmul per slot at the end
  epilogue: O = transpose_bf16(O^T) * 1/(d*sqrt(H)) + reciprocal on DVE -> DMA
"""

import numpy as np
import ml_dtypes
from contextlib import ExitStack

import concourse.bass as bass
import concourse.tile as tile
from concourse import bacc, mybir
from concourse.bass_utils import run_bass_kernel_spmd

B, S, D, H = 4, 4096, 1024, 128
P = 128
BF16 = mybir.dt.bfloat16
F32 = mybir.dt.float32
NPBF16 = ml_dtypes.bfloat16

QLOC = 2048          # query rows per core
NSLOT = 4            # slots per core
SLOT_W = 512         # q columns per slot
LIMITS = [8, 16, 24, 32]   # k-tile limit per slot (same for every core)
NKT = S // P         # 32 k tiles
DCH = D // P         # 8 contraction chunks
SCALE = 1.0 / float(np.sqrt(H))     # pre-exp scale
POSTSCALE = float(np.sqrt(H))       # folded into denominator


def qglob_for_core(h):
    """Global query row indices (length QLOC) handled by core-half h, in local order."""
    idx = []
    for r in range(NSLOT):
        for s in range(4):
            g = 8 * r + 2 * s + h
            idx.append(np.arange(g * P, (g + 1) * P))
    return np.concatenate(idx)


def build_nc():
    nc = bacc.Bacc(None, target_bir_lowering=False, debug=False, num_devices=8)

    xt = nc.dram_tensor("xt", [D, S], BF16, kind="ExternalInput").ap()
    xqt = nc.dram_tensor("xqt", [D, QLOC], BF16, kind="ExternalInput").ap()
    w_ap = {}
    for nm in ("wq", "wk", "wv"):
        w_ap[nm] = nc.dram_tensor(nm, [D, H], BF16, kind="ExternalInput").ap()
    b_ap = {}
    for nm in ("bq", "bk", "bv"):
        b_ap[nm] = nc.dram_tensor(nm, [H, 1], F32, kind="ExternalInput").ap()
    qpos = nc.dram_tensor("qpos", [1, QLOC], mybir.dt.int16, kind="ExternalInput").ap()
    kio = nc.dram_tensor("kio", [P, NKT], mybir.dt.int16, kind="ExternalInput").ap()
    identb = nc.dram_tensor("identb", [P, P], BF16, kind="ExternalInput").ap()
    identf = nc.dram_tensor("identf", [P, P], F32, kind="ExternalInput").ap()
    onesb = nc.dram_tensor("onesb", [P, 1], BF16, kind="ExternalInput").ap()
    out = nc.dram_tensor("out", [QLOC, H], F32, kind="ExternalOutput").ap()

    Ident = mybir.ActivationFunctionType.Identity
    Copy = mybir.ActivationFunctionType.Copy
    Exp = mybir.ActivationFunctionType.Exp

    with tile.TileContext(nc) as tc, ExitStack() as ctx:
        consts = ctx.enter_context(tc.tile_pool(name="consts", bufs=1))
        persist = ctx.enter_context(tc.tile_pool(name="persist", bufs=1))

        # ---- constants into SBUF
        w_sb = {}
        for nm in ("wq", "wk", "wv"):
            t = consts.tile([P, DCH, H], BF16, tag=f"w_{nm}")
            nc.sync.dma_start(out=t[:], in_=w_ap[nm].rearrange("(c p) h -> p c h", p=P))
            w_sb[nm] = t
        b_sb = {}
        for nm in ("bq", "bk", "bv"):
            t = consts.tile([P, 1], F32, tag=f"b_{nm}")
            nc.sync.dma_start(out=t[:], in_=b_ap[nm])
            b_sb[nm] = t
        qpos_b = consts.tile([P, QLOC], mybir.dt.int16, tag="qpos_b")
        nc.gpsimd.dma_start(
            out=qpos_b[:],
            in_=bass.AP(tensor=qpos.tensor, offset=qpos.offset, ap=[[0, P], [1, QLOC]]),
        )
        kio_sb = consts.tile([P, NKT], mybir.dt.int16, tag="kio")
        nc.sync.dma_start(out=kio_sb[:], in_=kio)
        identf_sb = consts.tile([P, P], F32, tag="identf")
        nc.sync.dma_start(out=identf_sb[:], in_=identf)
        identb_sb = consts.tile([P, P], BF16, tag="identb")
        nc.sync.dma_start(out=identb_sb[:], in_=identb)
        ones_sb = consts.tile([P, 1], BF16, tag="ones")
        nc.sync.dma_start(out=ones_sb[:], in_=onesb)

        # ---- persistent activations
        kT = persist.tile([P, S], BF16, tag="kT")          # K^T [h, s]
        vN = persist.tile([P, NKT, H], BF16, tag="vN")     # V natural [k_l, kt, h]
        qT = persist.tile([P, QLOC], BF16, tag="qT")       # Q^T [h, q_local]
        xt_sb = persist.tile([P, DCH, S], BF16, tag="xt_sb")    # x^T resident
        xqt_sb = persist.tile([P, DCH, QLOC], BF16, tag="xqt_sb")

        # PSUM budget (8 banks): mm512 x3 + sT x3 + oT-pair x1(2 banks) = 8
        with tc.tile_pool(name="stg", bufs=3) as stg, \
             tc.tile_pool(name="mm", bufs=3, space="PSUM") as psA, \
             tc.tile_pool(name="psS", bufs=3, space="PSUM") as psS, \
             tc.tile_pool(name="psO", bufs=1, space="PSUM") as psO, \
             tc.tile_pool(name="pp", bufs=8) as pp, \
             tc.tile_pool(name="acc", bufs=1) as accp, \
             tc.tile_pool(name="epi", bufs=3) as epi:

            # issue x^T loads column-range-major so the first stripes' full
            # contraction (all 8 chunks) lands as early as possible
            def load_xt_cols(c0, c1):
                for j in range(DCH):
                    nc.sync.dma_start(
                        out=xt_sb[:, j, c0:c1],
                        in_=xt[j * P:(j + 1) * P, c0:c1],
                    )

            def load_xqt_cols(c0, c1):
                for j in range(DCH):
                    nc.sync.dma_start(
                        out=xqt_sb[:, j, c0:c1],
                        in_=xqt[j * P:(j + 1) * P, c0:c1],
                    )

            Q4 = S // 4
            load_xt_cols(0, SLOT_W)          # small first block: PE starts sooner
            load_xt_cols(SLOT_W, Q4)
            load_xqt_cols(0, QLOC // 2)      # early: attention pass 1 needs Q^T
            load_xt_cols(Q4, 2 * Q4)
            load_xqt_cols(QLOC // 2, QLOC)
            load_xt_cols(2 * Q4, 3 * Q4)
            load_xt_cols(3 * Q4, S)

            def project_pair(src_sb, srs, wname):
                """Weight-stationary projection of a pair of 512-col stripes."""
                pss = [psA.tile([P, SLOT_W], F32, tag="mm512", name=f"prj{i}")
                       for i in range(len(srs))]
                for j in range(DCH):
                    for i, sr in enumerate(srs):
                        nc.tensor.matmul(
                            pss[i][:], lhsT=w_sb[wname][:, j, :],
                            rhs=src_sb[:, j, sr * SLOT_W:(sr + 1) * SLOT_W],
                            start=(j == 0), stop=(j == DCH - 1),
                        )
                return pss

            def kv_stripes(*srs):
                """K^T, V for the given 512-col stripes."""
                for ps, sr in zip(project_pair(xt_sb, srs, "wk"), srs):
                    nc.scalar.activation(
                        kT[:, sr * SLOT_W:(sr + 1) * SLOT_W], ps[:], Ident,
                        bias=b_sb["bk"][:], scale=1.0,
                    )
                for ps, sr in zip(project_pair(xt_sb, srs, "wv"), srs):
                    vTs = stg.tile([P, SLOT_W], BF16, tag="vT")
                    nc.scalar.activation(vTs[:], ps[:], Ident, bias=b_sb["bv"][:], scale=1.0)
                    pst = psA.tile([P, SLOT_W], BF16, tag="mm512", name="vtr")
                    for t_ in range(4):
                        nc.tensor.matmul(
                            pst[:, t_ * P:(t_ + 1) * P], lhsT=vTs[:, t_ * P:(t_ + 1) * P],
                            rhs=identb_sb[:], is_transpose=True, skip_group_check=True,
                        )
                    nc.vector.tensor_copy(vN[:, sr * 4:(sr + 1) * 4, :], pst[:])

            def q_stripes(*srs):
                for ps, qr in zip(project_pair(xqt_sb, srs, "wq"), srs):
                    nc.scalar.activation(
                        qT[:, qr * SLOT_W:(qr + 1) * SLOT_W], ps[:], Ident,
                        bias=b_sb["bq"][:], scale=1.0,
                    )

            def attention_pass(slots):
                """kt-outer attention over a pair of slots (shared K/V bricks)."""
                Ls = [LIMITS[r] for r in slots]
                Lmax = max(Ls)
                oT = psO.tile([P, len(slots), SLOT_W], F32, tag="oT")
                dacc = {r: accp.tile([P, SLOT_W], BF16, tag=f"dacc{r}", name=f"dacc{r}")
                        for r in slots}

                def c0_of(r, kt):
                    # first column (h-safe) that any core's subtile can still
                    # attend at this k-brick; earlier columns are masked for
                    # both halves and are skipped entirely
                    return P * max(0, (kt - 8 * r) // 2)

                def score(kt):
                    """S^T + exp (+ causal mask) for every slot active at kt."""
                    pTs = {}
                    for r in slots:
                        if kt >= LIMITS[r]:
                            continue
                        c0 = c0_of(r, kt)
                        qsl = slice(r * SLOT_W + c0, (r + 1) * SLOT_W)
                        sT = psS.tile([P, SLOT_W], F32, tag="sT")
                        nc.tensor.matmul(
                            sT[:, c0:], lhsT=kT[:, kt * P:(kt + 1) * P], rhs=qT[:, qsl],
                            start=True, stop=True,
                        )
                        pT = pp.tile([P, SLOT_W], BF16, tag="pT")
                        nc.scalar.activation(pT[:, c0:], sT[:, c0:], Exp, scale=SCALE)
                        if kt >= LIMITS[r] - 8:
                            # pT = (qpos >= kpos) * pT  (fused causal mask)
                            nc.vector.scalar_tensor_tensor(
                                pT[:, c0:], qpos_b[:, qsl], kio_sb[:, kt:kt + 1],
                                pT[:, c0:],
                                op0=mybir.AluOpType.is_ge, op1=mybir.AluOpType.mult,
                            )
                        # denominator partial sums on DVE (bf16)
                        if kt == 0:
                            nc.vector.tensor_copy(dacc[r][:], pT[:])
                        else:
                            nc.vector.tensor_add(
                                dacc[r][:, c0:], dacc[r][:, c0:], pT[:, c0:]
                            )
                        pTs[r] = (pT, c0)
                    return pTs

                def accum(kt, pTs):
                    for i, r in enumerate(slots):
                        if r in pTs:
                            pT, c0 = pTs[r]
                            nc.tensor.matmul(
                                oT[:, i, c0:], lhsT=vN[:, kt, :], rhs=pT[:, c0:],
                                start=(kt == 0), stop=(kt == LIMITS[r] - 1),
                            )

                pT_prev = score(0)
                for kt in range(1, Lmax):
                    pTs = score(kt)
                    accum(kt - 1, pT_prev)
                    pT_prev = pTs
                accum(Lmax - 1, pT_prev)

                # epilogue per slot: O = transpose(O^T) / (d * sqrt(H))
                for i, r in enumerate(slots):
                    d_ps = psA.tile([1, SLOT_W], F32, tag="mm512")
                    nc.tensor.matmul(
                        d_ps[:], lhsT=ones_sb[:], rhs=dacc[r][:],
                        start=True, stop=True,
                    )
                    oTs = epi.tile([P, SLOT_W], BF16, tag="oTs")
                    nc.scalar.activation(oTs[:], oT[:, i, :], Copy)
                    ds_ = epi.tile([1, SLOT_W], F32, tag="ds")
                    nc.scalar.activation(ds_[:], d_ps[:], Copy, scale=POSTSCALE)
                    dT = psA.tile([P, 4], F32, tag="mm512")
                    for s_ in range(4):
                        nc.tensor.matmul(
                            dT[:, s_:s_ + 1], lhsT=ds_[:, s_ * P:(s_ + 1) * P],
                            rhs=identf_sb[0:1, 0:1], is_transpose=True,
                            skip_group_check=True,
                        )
                    rec = epi.tile([P, 4], F32, tag="rec")
                    nc.vector.reciprocal(rec[:], dT[:])
                    obr = psA.tile([P, SLOT_W], BF16, tag="mm512", name="obr")
                    for s_ in range(4):
                        nc.tensor.matmul(
                            obr[:, s_ * P:(s_ + 1) * P], lhsT=oTs[:, s_ * P:(s_ + 1) * P],
                            rhs=identb_sb[:], is_transpose=True, skip_group_check=True,
                        )
                    ofin = epi.tile([P, SLOT_W], F32, tag="ofin")
                    for s_ in range(4):
                        nc.vector.tensor_scalar_mul(
                            ofin[:, s_ * P:(s_ + 1) * P], obr[:, s_ * P:(s_ + 1) * P],
                            rec[:, s_:s_ + 1],
                        )
                    nc.sync.dma_start(
                        out=out[r * SLOT_W:(r + 1) * SLOT_W, :].rearrange(
                            "(s p) h -> p s h", p=P
                        ),
                        in_=ofin[:].rearrange("p (s h) -> p s h", s=4),
                    )

            # emission: all projections first (their PSUM slot allocations must
            # not queue behind pass epilogues), then the attention passes
            kv_stripes(0, 1)
            q_stripes(0, 1)
            kv_stripes(2, 3)
            kv_stripes(4, 5)
            kv_stripes(6, 7)
            q_stripes(2, 3)
            attention_pass((0, 1))
            attention_pass((2, 3))

    nc.compile()
    return nc


_NC_CACHE = None


def _get_nc():
    global _NC_CACHE
    if _NC_CACHE is None:
        _NC_CACHE = build_nc()
    return _NC_CACHE


def make_in_maps(inputs):
    x = np.asarray(inputs["x"], np.float32)
    Wq = np.asarray(inputs["Wq"], np.float32)
    Wk = np.asarray(inputs["Wk"], np.float32)
    Wv = np.asarray(inputs["Wv"], np.float32)
    bq = np.asarray(inputs["bq"], np.float32)
    bk = np.asarray(inputs["bk"], np.float32)
    bv = np.asarray(inputs["bv"], np.float32)

    xb = x.astype(NPBF16)
    kio = (np.arange(NKT)[None, :] * P + np.arange(P)[:, None]).astype(np.int16)
    common = dict(
        wq=Wq.astype(NPBF16), wk=Wk.astype(NPBF16), wv=Wv.astype(NPBF16),
        bq=bq.reshape(H, 1), bk=bk.reshape(H, 1), bv=bv.reshape(H, 1),
        kio=kio,
        identb=np.eye(P, dtype=NPBF16),
        identf=np.eye(P, dtype=np.float32),
        onesb=np.ones((P, 1), dtype=NPBF16),
    )
    in_maps = []
    xbT = np.ascontiguousarray(xb.transpose(0, 2, 1))  # [B, D, S]
    for c in range(8):
        b, hh = c // 2, c % 2
        qg = qglob_for_core(hh)
        m = dict(common)
        m["xt"] = xbT[b]
        m["xqt"] = np.ascontiguousarray(xbT[b][:, qg])
        m["qpos"] = qg.astype(np.int16).reshape(1, QLOC)
        in_maps.append(m)
    return in_maps


def assemble_out(results):
    out = np.zeros((1, B, S, H), np.float32)
    for c in range(8):
        b, hh = c // 2, c % 2
        qg = qglob_for_core(hh)
        out[0, b, qg, :] = results[c]["out"]
    return out


def kernel(**inputs) -> np.ndarray:
    nc = _get_nc()
    in_maps = make_in_maps(inputs)
    res = run_bass_kernel_spmd(nc, in_maps, list(range(8)))
    return assemble_out(res.results)

